# revision 9
# baseline (speedup 1.0000x reference)
"""GAT (2-layer, 4-head) Bass kernel for Trainium2, data-parallel over 8 NeuronCores.

Math (per sample b, per attention instance with weights W, a = [a1; a2]):
    Wh = h @ W                      [N, F]
    s  = Wh @ a1   (per-dst-node i score part)
    t  = Wh @ a2   (per-src-node j score part)
    e[i,j]   = leaky_relu(s[i] + t[j], 0.2)
    att      = softmax_j(where(adj[i,j] > 0, e, -9e15))
    out[i]   = sum_j att[i,j] * Wh[j]

Kernel layout choices:
  - All N x N score/attention tiles live as [j(part), i(free)] "transposed"
    tiles so the final contraction over j runs on the PE with pT chunks as
    stationary (lhsT) operands, and so that exp's affine pre-add can fuse
    s (free-broadcast via a PE rank-1 matmul) with t (per-partition bias).
  - leaky_relu+exp is exact:  exp(lrelu(z)) needs two ACT passes (Lrelu, Exp);
    for a subset of j-tiles the lrelu runs on DVE instead as
    z = S + t ; z02 = 0.2*S + 0.2*t ; L = max(z, z02)   (fp16, modes 4x/4x/2x)
    to balance ACT vs DVE occupancy.
  - Masking is one DVE tensor_tensor mult with a host-pre-transposed bf16
    0/1 mask (exactly equivalent to the -9e15 additive mask: rows always
    have >= 1 edge at this density, and softmax normalization cancels the
    missing max-subtraction; values |z| <~ 15 are safe in fp32 exp).
  - Softmax row-sums come for free as a ones-column appended to the Wh
    matmul rhs; normalization is a per-partition reciprocal + scale of the
    small [128, 65] outputs.
"""

import os
import sys

import numpy as np

if not os.path.isdir(os.path.join(os.path.dirname(os.path.abspath(__file__)), "concourse")):
    for _p in ("/opt/trn_rl_repo", os.path.expanduser("~/.axon_site/_ro/trn_rl_repo")):
        if os.path.isdir(_p) and _p not in sys.path:
            sys.path.append(_p)

import ml_dtypes  # noqa: E402

import concourse.bass as bass  # noqa: E402
import concourse.bacc as bacc  # noqa: E402
import concourse.tile as tile  # noqa: E402
from concourse import mybir  # noqa: E402
from concourse.bass_utils import run_bass_kernel_spmd  # noqa: E402

BF16 = ml_dtypes.bfloat16

B, N, FIN, FH, H, FOUT = 16, 1024, 256, 64, 4, 64
NCORES = 8
SPC = B // NCORES  # samples per core
KT = FIN // 128    # k tiles (2)
JT = N // 128      # j tiles (8)
ALPHA = 0.2

# j-tiles [0, NACT) run lrelu on ScalarE; the rest build it on VectorE.
NACT = 4

F32 = mybir.dt.float32
F16 = mybir.dt.float16
BF = mybir.dt.bfloat16
AF = mybir.ActivationFunctionType
OP = mybir.AluOpType


def _gat_instance(nc, pools, io, s, inst):
    """One attention instance (one head of L1, or L2) for sample s.

    inst dict:
      rep(kt)   -> AP [128,128] bf16  stationary Wa1-replicated (S matmul lhsT)
      rhs(kt)   -> AP [128,1024] bf16 moving input rows^T (x^T or h_cat^T)
      wh(jt)    -> AP [128,65]  bf16  [Wh block | ones col] att-matmul weights
      tcol(jt)  -> AP [128,1] f32    t bias column
      t02(jt)   -> AP [128,1] f16    0.2*t in fp16 (DVE path)
      emit(ic, o_nb) -> None        consume normalized bf16 [128,64] output tile
    """
    work, psA, psO = pools["work"], pools["psA"], pools["psO"]

    # S_bcast[p, i] = s[i] for all p: rank-K matmul, K=256 contraction with
    # every lhsT column equal to Wa1.
    sb_ps = psA.tile([128, N], F32, tag="big")
    for kt in range(KT):
        for ih in range(2):
            nc.tensor.matmul(
                sb_ps[:, ih * 512 : (ih + 1) * 512],
                inst["rep"](kt),
                inst["rhs"](kt)[:, ih * 512 : (ih + 1) * 512],
                start=(kt == 0),
                stop=(kt == KT - 1),
            )
    sb16 = work.tile([128, N], F16, tag="sb16")
    nc.vector.tensor_copy(sb16, sb_ps)

    pT = work.tile([128, JT, N], BF, tag="pt")
    for jt in range(JT):
        p = work.tile([128, N], BF, tag="p")
        if jt < NACT:
            # exp(lrelu(z)) == max(exp(z), exp(0.2 z)) -- two ACT exps with the
            # z = S_bcast + t affine folded in, then a DVE max.
            e1 = work.tile([128, N], BF, tag="e1")
            nc.scalar.activation(e1, sb_ps, AF.Exp, bias=inst["tcol"](jt), scale=1.0)
            e2 = work.tile([128, N], BF, tag="e2")
            nc.scalar.activation(e2, sb_ps, AF.Exp, bias=inst["t02"](jt), scale=ALPHA)
            nc.vector.tensor_tensor(p, e1, e2, OP.max)
        else:
            # z-space lrelu on DVE (fp16), then one ACT exp.
            z = work.tile([128, N], F16, tag="z")
            nc.vector.tensor_scalar(z, sb16, inst["tcol"](jt), None, OP.add)
            z02 = work.tile([128, N], F16, tag="z02")
            nc.vector.tensor_scalar(z02, sb16, ALPHA, inst["t02"](jt), OP.mult, OP.add)
            ell = work.tile([128, N], F16, tag="ell")
            nc.vector.tensor_tensor(ell, z, z02, OP.max)
            nc.scalar.activation(p, ell, AF.Exp)
        nc.vector.tensor_tensor(pT[:, jt, :], p, io["maskT_sb"][:, jt, :], OP.mult)

    for ic in range(JT):
        o_ps = psO.tile([128, FH + 1], F32, tag="o")
        for jt in range(JT):
            nc.tensor.matmul(
                o_ps,
                pT[:, jt, ic * 128 : (ic + 1) * 128],
                inst["wh"](jt),
                start=(jt == 0),
                stop=(jt == JT - 1),
            )
        rcol = work.tile([128, 1], F32, tag="rcol")
        nc.vector.reciprocal(rcol, o_ps[:, FH : FH + 1])
        o_nb = work.tile([128, FH], BF, tag="onb")
        nc.vector.tensor_scalar(o_nb, o_ps[:, 0:FH], rcol, None, OP.mult)
        inst["emit"](ic, o_nb)


def _build_nc():
    nc = bacc.Bacc()

    xT_d = nc.declare_dram_parameter("xT", [SPC, KT, 128, N], BF, isOutput=False)
    maskT_d = nc.declare_dram_parameter("maskT", [SPC, JT, 128, N], BF, isOutput=False)
    wbig1_d = nc.declare_dram_parameter("wbig1", [KT, 128, H * 65 + H], BF, isOutput=False)
    warep1_d = nc.declare_dram_parameter("warep1", [KT, 128, H * 128], BF, isOutput=False)
    wbig2_d = nc.declare_dram_parameter("wbig2", [KT, 128, 66], BF, isOutput=False)
    warep2_d = nc.declare_dram_parameter("warep2", [KT, 128, 128], BF, isOutput=False)
    ident_d = nc.declare_dram_parameter("ident", [128, 128], BF, isOutput=False)
    out_d = nc.declare_dram_parameter("out", [SPC, FOUT], F32, isOutput=True)

    with tile.TileContext(nc) as tc:
        with (
            tc.tile_pool(name="const", bufs=1) as constp,
            tc.tile_pool(name="samp", bufs=2) as samp,
            tc.tile_pool(name="work", bufs=3) as work,
            tc.tile_pool(name="psA", bufs=2, space="PSUM") as psA,
            tc.tile_pool(name="psO", bufs=2, space="PSUM") as psO,
            tc.tile_pool(name="psT", bufs=1, space="PSUM") as psT,
            tc.tile_pool(name="psM", bufs=1, space="PSUM") as psM,
        ):
            pools = {"work": work, "psA": psA, "psO": psO}

            wbig1_sb = constp.tile([128, KT, H * 65 + H], BF)
            warep1_sb = constp.tile([128, KT, H * 128], BF)
            wbig2_sb = constp.tile([128, KT, 66], BF)
            warep2_sb = constp.tile([128, KT, 128], BF)
            ident_sb = constp.tile([128, 128], BF)
            for kt in range(KT):
                nc.sync.dma_start(out=wbig1_sb[:, kt, :], in_=wbig1_d[kt])
                nc.sync.dma_start(out=warep1_sb[:, kt, :], in_=warep1_d[kt])
                nc.sync.dma_start(out=wbig2_sb[:, kt, :], in_=wbig2_d[kt])
                nc.sync.dma_start(out=warep2_sb[:, kt, :], in_=warep2_d[kt])
            nc.sync.dma_start(out=ident_sb, in_=ident_d[:, :])
            meanw_sb = constp.tile([128, 1], BF)
            nc.vector.memset(meanw_sb, 1.0 / N)

            for s in range(SPC):
                xT_sb = samp.tile([128, KT, N], BF, tag="xt")
                for kt in range(KT):
                    nc.sync.dma_start(out=xT_sb[:, kt, :], in_=xT_d[s, kt])
                maskT_sb = samp.tile([128, JT, N], BF, tag="mask")
                for jt in range(JT):
                    nc.sync.dma_start(out=maskT_sb[:, jt, :], in_=maskT_d[s, jt])
                io = {"maskT_sb": maskT_sb}

                # ---- L1 Wh for all 4 heads (+ t columns) ----
                whsb1 = samp.tile([128, JT, H * 65], BF, tag="whsb1")
                tc1 = samp.tile([128, JT, H], F32, tag="tc1")
                t02_1 = samp.tile([128, JT, H], F32, tag="t02_1")
                for jt in range(JT):
                    wm_ps = psA.tile([128, H * 65 + H], F32, tag="big")
                    for kt in range(KT):
                        nc.tensor.matmul(
                            wm_ps,
                            xT_sb[:, kt, jt * 128 : (jt + 1) * 128],
                            wbig1_sb[:, kt, :],
                            start=(kt == 0),
                            stop=(kt == KT - 1),
                        )
                    eng = nc.vector if jt % 2 == 0 else nc.scalar
                    if jt % 2 == 0:
                        nc.vector.tensor_copy(whsb1[:, jt, :], wm_ps[:, 0 : H * 65])
                    else:
                        nc.scalar.copy(whsb1[:, jt, :], wm_ps[:, 0 : H * 65])
                    nc.vector.memset(whsb1[:, jt, FH : H * 65 : 65], 1.0)
                    nc.vector.tensor_copy(tc1[:, jt, :], wm_ps[:, H * 65 : H * 65 + H])
                    nc.vector.tensor_scalar(
                        t02_1[:, jt, :], wm_ps[:, H * 65 : H * 65 + H], ALPHA, None, OP.mult
                    )

                # ---- L1 attention, 4 heads -> h_cat^T ----
                hcatT = samp.tile([128, KT, N], BF, tag="hcat")

                def mk_emit_l1(h):
                    def emit(ic, o_nb):
                        tp_ps = psT.tile([64, 128], BF, tag="tr")
                        nc.tensor.transpose(tp_ps, o_nb, ident_sb)
                        dst = hcatT[
                            (h % 2) * 64 : (h % 2) * 64 + 64, h // 2, ic * 128 : (ic + 1) * 128
                        ]
                        if ic % 2 == 0:
                            nc.vector.tensor_copy(dst, tp_ps)
                        else:
                            nc.scalar.copy(dst, tp_ps)

                    return emit

                for h in range(H):
                    _gat_instance(
                        nc,
                        pools,
                        io,
                        s,
                        {
                            "rep": lambda kt, h=h: warep1_sb[:, kt, h * 128 : (h + 1) * 128],
                            "rhs": lambda kt: xT_sb[:, kt, :],
                            "wh": lambda jt, h=h: whsb1[:, jt, h * 65 : (h + 1) * 65],
                            "tcol": lambda jt, h=h: tc1[:, jt, h : h + 1],
                            "t02": lambda jt, h=h: t02_1[:, jt, h : h + 1],
                            "emit": mk_emit_l1(h),
                        },
                    )

                # ---- L2 Wh ----
                whsb2 = samp.tile([128, JT, 65], BF, tag="whsb2")
                tc2 = samp.tile([128, JT, 1], F32, tag="tc2")
                t02_2 = samp.tile([128, JT, 1], F32, tag="t02_2")
                for jt in range(JT):
                    wm_ps = psA.tile([128, 66], F32, tag="big")
                    for kt in range(KT):
                        nc.tensor.matmul(
                            wm_ps,
                            hcatT[:, kt, jt * 128 : (jt + 1) * 128],
                            wbig2_sb[:, kt, :],
                            start=(kt == 0),
                            stop=(kt == KT - 1),
                        )
                    nc.vector.tensor_copy(whsb2[:, jt, 0:FOUT], wm_ps[:, 0:FOUT])
                    nc.vector.memset(whsb2[:, jt, FOUT : FOUT + 1], 1.0)
                    nc.vector.tensor_copy(tc2[:, jt, :], wm_ps[:, 65:66])
                    nc.vector.tensor_scalar(t02_2[:, jt, :], wm_ps[:, 65:66], ALPHA, None, OP.mult)

                # ---- L2 attention + elu + mean ----
                elu_sb = samp.tile([128, JT, FOUT], BF, tag="elu2")

                def emit_l2(ic, o_nb):
                    # elu(x) = max(x,0) + min(exp(x)-1, 0)
                    ex = work.tile([128, FOUT], F32, tag="ex")
                    nc.scalar.activation(ex, o_nb, AF.Exp)
                    bmax = work.tile([128, FOUT], BF, tag="bmax")
                    nc.vector.tensor_scalar(bmax, o_nb, 0.0, None, OP.max)
                    cmin = work.tile([128, FOUT], BF, tag="cmin")
                    nc.vector.tensor_scalar(cmin, ex, -1.0, 0.0, OP.add, OP.min)
                    nc.vector.tensor_tensor(elu_sb[:, ic, :], bmax, cmin, OP.add)

                _gat_instance(
                    nc,
                    pools,
                    io,
                    s,
                    {
                        "rep": lambda kt: warep2_sb[:, kt, :],
                        "rhs": lambda kt: hcatT[:, kt, :],
                        "wh": lambda jt: whsb2[:, jt, :],
                        "tcol": lambda jt: tc2[:, jt, :],
                        "t02": lambda jt: t02_2[:, jt, :],
                        "emit": emit_l2,
                    },
                )

                # mean over nodes via ones/N matmul, then DMA the [64,1] column
                mean_ps = psM.tile([FOUT, 1], F32, tag="mean")
                for ic in range(JT):
                    nc.tensor.matmul(
                        mean_ps, elu_sb[:, ic, :], meanw_sb,
                        start=(ic == 0), stop=(ic == JT - 1),
                    )
                mean_sb = work.tile([FOUT, 1], F32, tag="meansb")
                nc.vector.tensor_copy(mean_sb, mean_ps)
                nc.sync.dma_start(
                    out=out_d[s].rearrange("(f a) -> f a", a=1), in_=mean_sb
                )

    nc.finalize()
    return nc


_NC_CACHE = None


def _prep_host(x, adj, W_heads, a_heads, W_out, a_out):
    xT = np.ascontiguousarray(x.transpose(0, 2, 1)).astype(BF16)  # [B, FIN, N]
    xT = xT.reshape(B, KT, 128, N)
    maskT = (adj > 0).transpose(0, 2, 1).astype(BF16)  # [B, j, i]
    maskT = np.ascontiguousarray(maskT).reshape(B, JT, 128, N)

    wbig1 = np.zeros((FIN, H * 65 + H), dtype=np.float32)
    warep1 = np.zeros((FIN, H * 128), dtype=np.float32)
    for h in range(H):
        Wh_ = W_heads[h].astype(np.float32)
        a1 = a_heads[h, :FH, 0].astype(np.float32)
        a2 = a_heads[h, FH:, 0].astype(np.float32)
        wbig1[:, h * 65 : h * 65 + FH] = Wh_
        wbig1[:, H * 65 + h] = Wh_ @ a2
        warep1[:, h * 128 : (h + 1) * 128] = (Wh_ @ a1)[:, None]
    wbig2 = np.zeros((FIN, 66), dtype=np.float32)
    wbig2[:, 0:FOUT] = W_out.astype(np.float32)
    wbig2[:, 65] = W_out.astype(np.float32) @ a_out[FOUT:, 0].astype(np.float32)
    warep2 = np.repeat(
        (W_out.astype(np.float32) @ a_out[:FOUT, 0].astype(np.float32))[:, None], 128, axis=1
    )

    shared = {
        "wbig1": wbig1.astype(BF16).reshape(KT, 128, H * 65 + H),
        "warep1": warep1.astype(BF16).reshape(KT, 128, H * 128),
        "wbig2": wbig2.astype(BF16).reshape(KT, 128, 66),
        "warep2": warep2.astype(BF16).reshape(KT, 128, 128),
        "ident": np.eye(128, dtype=np.float32).astype(BF16),
    }
    in_maps = []
    for c in range(NCORES):
        sl = slice(c * SPC, (c + 1) * SPC)
        m = {"xT": np.ascontiguousarray(xT[sl]), "maskT": np.ascontiguousarray(maskT[sl])}
        m.update(shared)
        in_maps.append(m)
    return in_maps


def kernel(x, adj, W_heads, a_heads, W_out, a_out, _trace=False):
    global _NC_CACHE
    if _NC_CACHE is None:
        _NC_CACHE = _build_nc()
    nc = _NC_CACHE
    in_maps = _prep_host(x, adj, W_heads, a_heads, W_out, a_out)
    res = run_bass_kernel_spmd(nc, in_maps, core_ids=list(range(NCORES)), trace=_trace)
    out = np.concatenate([res.results[c]["out"] for c in range(NCORES)], axis=0)
    if _trace:
        kernel._last_results = res
    return out.astype(np.float32)
